# revision 1
# baseline (speedup 1.0000x reference)
"""DualMaskRoIPool Trainium2 kernel.

Strategy: shard the 64 ROIs across 8 NeuronCores (8 ROIs each, clustered by
union-box row range so each core only loads a row slice of the feature map).
The ROI coordinates are known when `kernel()` is called, so each core gets a
specialized straight-line Bass/Tile program:

  per ROI: ScalarE copies the union-box window of the feature map into an
  SBUF val buffer, GPSIMD memsets the dual-mask complement rectangles to 0
  (val == feat * mask exactly), and VectorE runs batched 5D max-reduces over
  the arithmetic runs of the adaptive 7x7 bin grid.  All fp32 max ops, so the
  result is bit-exact vs the reference.

The 8 per-core programs are dispatched concurrently to the 8 devices via the
bass2jax PJRT path.
"""

import numpy as np

PH, PW = 7, 7
SCALE = 0.0625
C, H, W = 128, 56, 56
NCORES = 8
NROIS = 64
RPC = NROIS // NCORES  # rois per core


# ----------------------------------------------------------------- geometry

def _zoom(rois):
    """Exact replica of the reference _zoom (fp32 scale, round-half-even)."""
    s = np.round(rois[:, 1:].astype(np.float32) * np.float32(SCALE)).astype(np.int32)
    x1 = np.where(s[:, 0] >= W, W - 1, s[:, 0])
    y1 = np.where(s[:, 1] >= H, H - 1, s[:, 1])
    x2 = np.where(s[:, 2] >= W, W - 1, s[:, 2])
    y2 = np.where(s[:, 3] >= H, H - 1, s[:, 3])
    return x1, y1, x2, y2


def _bin_edges(lo, extent):
    """Adaptive bin starts/lengths along one axis (absolute coords)."""
    starts = np.array([lo + (i * extent) // PH for i in range(PH)], np.int64)
    ends = np.array([lo + ((i + 1) * extent + PH - 1) // PH for i in range(PH)], np.int64)
    return starts, ends - starts


def _runs(starts, lens):
    """Split the 7 bins into maximal runs with uniform gap and length."""
    runs = []
    i = 0
    while i < PH:
        n = 1
        gap = 1
        while i + n < PH:
            g = int(starts[i + n] - starts[i + n - 1])
            if lens[i + n] != lens[i]:
                break
            if n == 1:
                gap = g
            elif g != gap:
                break
            n += 1
        runs.append((i, n, gap, int(lens[i])))
        i += n
    return runs


def _complement_rects(mask):
    """Maximal-row-band rectangles covering the False region of mask[h, w]."""
    h, w = mask.shape
    rects = []
    r = 0
    while r < h:
        r2 = r
        while r2 + 1 < h and np.array_equal(mask[r2 + 1], mask[r]):
            r2 += 1
        row = mask[r]
        x = 0
        while x < w:
            if not row[x]:
                x2 = x
                while x2 + 1 < w and not row[x2 + 1]:
                    x2 += 1
                rects.append((r, r2 + 1, x, x2 + 1))
                x = x2 + 1
            else:
                x += 1
        r = r2 + 1
    return rects


def _geometry(rois_1, rois_2):
    x1a, y1a, x2a, y2a = _zoom(np.asarray(rois_1))
    x1b, y1b, x2b, y2b = _zoom(np.asarray(rois_2))
    ux1 = np.minimum(x1a, x1b)
    uy1 = np.minimum(y1a, y1b)
    ux2 = np.maximum(x2a, x2b)
    uy2 = np.maximum(y2a, y2b)
    geoms = []
    for b in range(len(ux1)):
        lo_y, hi_y = int(uy1[b]), int(uy2[b])
        lo_x, hi_x = int(ux1[b]), int(ux2[b])
        h = hi_y - lo_y + 1
        w = hi_x - lo_x + 1
        mask = np.zeros((h, w), bool)
        mask[y1a[b] - lo_y:y2a[b] - lo_y + 1, x1a[b] - lo_x:x2a[b] - lo_x + 1] = True
        mask[y1b[b] - lo_y:y2b[b] - lo_y + 1, x1b[b] - lo_x:x2b[b] - lo_x + 1] = True
        rs, hgt = _bin_edges(lo_y, h)
        cs, wdt = _bin_edges(lo_x, w)
        geoms.append(dict(
            uy1=lo_y, uy2=hi_y, ux1=lo_x, ux2=hi_x, h=h, w=w,
            rects=_complement_rects(mask),
            iruns=_runs(rs, hgt), jruns=_runs(cs, wdt),
            rs=rs, cs=cs,
        ))
    return geoms


# ------------------------------------------------------------ program build

def _build_core_program(geoms, ylo, nrows):
    """Build the specialized Bacc program for one core (RPC rois)."""
    import concourse.bacc as bacc
    import concourse.bass as bass
    import concourse.tile as tile
    from concourse import mybir

    f32 = mybir.dt.float32
    nc = bacc.Bacc("TRN2", target_bir_lowering=False, debug=False)
    feat_d = nc.dram_tensor("feat", [C, nrows * W], f32, kind="ExternalInput").ap()
    out_d = nc.dram_tensor("out", [C, RPC * PH * PW], f32, kind="ExternalOutput").ap()

    maxhw = max(g["h"] * g["w"] for g in geoms)

    def sub_ap(base, off, dims):
        p0 = list(list(base.ap)[0])
        return bass.AP(base.tensor, base.offset + off, [p0] + [list(d) for d in dims])

    with tile.TileContext(nc) as tc:
        with tc.tile_pool(name="main", bufs=1) as pool, \
             tc.tile_pool(name="vals", bufs=4) as vpool:
            feat_t = pool.tile([C, nrows * W], f32)
            o_t = pool.tile([C, RPC * PH * PW], f32)
            nc.sync.dma_start(feat_t[:], feat_d[:])
            for k, g in enumerate(geoms):
                h, w = g["h"], g["w"]
                if g["rects"]:
                    vt = vpool.tile([C, maxhw], f32, tag="v")
                    # window copy on ScalarE
                    win = sub_ap(feat_t[:], (g["uy1"] - ylo) * W + g["ux1"],
                                 [[W, h], [1, w]])
                    nc.scalar.copy(
                        vt[:, 0:h * w].rearrange("p (a b) -> p a b", a=h), win)
                    # zero the mask complement on GPSIMD
                    for (r0, r1, c0, c1) in g["rects"]:
                        nc.gpsimd.memset(
                            sub_ap(vt[:], r0 * w + c0, [[w, r1 - r0], [1, c1 - c0]]),
                            0.0)
                    src, pitch, oy, ox = vt[:], w, g["uy1"], g["ux1"]
                else:
                    # mask covers the whole union box: reduce straight from feat
                    src, pitch, oy, ox = feat_t[:], W, ylo, 0
                for (i0, ni, gi, hgt) in g["iruns"]:
                    for (j0, nj, gj, wdt) in g["jruns"]:
                        in_ap = sub_ap(
                            src,
                            (int(g["rs"][i0]) - oy) * pitch + (int(g["cs"][j0]) - ox),
                            [[gi * pitch, ni], [gj, nj], [pitch, hgt], [1, wdt]])
                        out_ap = sub_ap(
                            o_t[:], k * PH * PW + i0 * PW + j0,
                            [[PW, ni], [1, nj]])
                        nc.vector.tensor_reduce(
                            out_ap, in_ap,
                            axis=mybir.AxisListType.XY, op=mybir.AluOpType.max)
            nc.sync.dma_start(out_d[:], o_t[:])
    nc.compile()
    return nc


# ---------------------------------------------------------------- top level

def _prepare(feature_map, rois_1, rois_2):
    """Returns (programs, in_maps, core_roi_ids)."""
    geoms = _geometry(rois_1, rois_2)
    order = sorted(range(NROIS), key=lambda b: geoms[b]["uy1"] + geoms[b]["uy2"])
    fm = np.ascontiguousarray(np.asarray(feature_map), np.float32)[0]  # [C,H,W]
    programs, in_maps, core_ids = [], [], []
    for c in range(NCORES):
        ids = order[c * RPC:(c + 1) * RPC]
        core_geoms = [geoms[b] for b in ids]
        ylo = min(g["uy1"] for g in core_geoms)
        yhi = max(g["uy2"] for g in core_geoms) + 1
        nrows = yhi - ylo
        programs.append(_build_core_program(core_geoms, ylo, nrows))
        in_maps.append({"feat": np.ascontiguousarray(
            fm[:, ylo:yhi, :]).reshape(C, nrows * W)})
        core_ids.append(ids)
    return programs, in_maps, core_ids


def _assemble(outs, core_ids):
    full = np.empty((NROIS, C, PH, PW), np.float32)
    for c in range(NCORES):
        r = outs[c]["out"].reshape(C, RPC, PH, PW).transpose(1, 0, 2, 3)
        for k, b in enumerate(core_ids[c]):
            full[b] = r[k]
    return full


def kernel(feature_map, rois_1, rois_2):
    import jax
    from concourse import bass2jax
    from concurrent.futures import ThreadPoolExecutor

    programs, in_maps, core_ids = _prepare(feature_map, rois_1, rois_2)
    bass2jax.install_neuronx_cc_hook()
    devices = jax.devices()

    def run_one(c):
        with jax.default_device(devices[c]):
            return bass2jax.run_bass_via_pjrt(programs[c], [in_maps[c]], n_cores=1)[0]

    with ThreadPoolExecutor(NCORES) as ex:
        outs = list(ex.map(run_one, range(NCORES)))
    return _assemble(outs, core_ids)


# revision 2
# speedup vs baseline: 1.1340x; 1.1340x over previous
"""DualMaskRoIPool Trainium2 kernel.

Strategy: shard the 64 ROIs across 8 NeuronCores, clustered by union-box row
range (each core only DMAs a row slice of the feature map) and balanced by
estimated compute cost.  ROI coordinates are known when `kernel()` runs, so
each core gets a specialized straight-line Bass/Tile program:

  per ROI: ScalarE copies the union-box window into an SBUF val buffer,
  GPSIMD memsets the dual-mask complement rectangles to 0 (val == feat*mask
  exactly), and VectorE max-reduces the adaptive 7x7 bin grid, either as a
  single multi-dim reduce per (row-run x col-run) of the grid or as a
  two-stage x-then-y pooling, whichever needs fewer cycles.  All-fp32 max
  ops -> bit-exact vs the reference.

The 8 per-core programs are dispatched concurrently to the 8 devices via the
bass2jax PJRT path.
"""

import numpy as np

PH, PW = 7, 7
SCALE = 0.0625
C, H, W = 128, 56, 56
NCORES = 8
NROIS = 64
DMA_CHUNKS = 4


# ----------------------------------------------------------------- geometry

def _zoom(rois):
    """Exact replica of the reference _zoom (fp32 scale, round-half-even)."""
    s = np.round(rois[:, 1:].astype(np.float32) * np.float32(SCALE)).astype(np.int32)
    x1 = np.where(s[:, 0] >= W, W - 1, s[:, 0])
    y1 = np.where(s[:, 1] >= H, H - 1, s[:, 1])
    x2 = np.where(s[:, 2] >= W, W - 1, s[:, 2])
    y2 = np.where(s[:, 3] >= H, H - 1, s[:, 3])
    return x1, y1, x2, y2


def _bin_edges(lo, extent):
    starts = np.array([lo + (i * extent) // PH for i in range(PH)], np.int64)
    ends = np.array([lo + ((i + 1) * extent + PH - 1) // PH for i in range(PH)], np.int64)
    return starts, ends - starts


def _runs(starts, lens):
    """Split the 7 bins into maximal runs with uniform gap and length."""
    runs = []
    i = 0
    while i < PH:
        n = 1
        gap = 1
        while i + n < PH:
            g = int(starts[i + n] - starts[i + n - 1])
            if lens[i + n] != lens[i]:
                break
            if n == 1:
                gap = g
            elif g != gap:
                break
            n += 1
        runs.append((i, n, gap, int(lens[i])))
        i += n
    return runs


def _complement_rects(mask):
    h, w = mask.shape
    rects = []
    r = 0
    while r < h:
        r2 = r
        while r2 + 1 < h and np.array_equal(mask[r2 + 1], mask[r]):
            r2 += 1
        row = mask[r]
        x = 0
        while x < w:
            if not row[x]:
                x2 = x
                while x2 + 1 < w and not row[x2 + 1]:
                    x2 += 1
                rects.append((r, r2 + 1, x, x2 + 1))
                x = x2 + 1
            else:
                x += 1
        r = r2 + 1
    return rects


def _geometry(rois_1, rois_2):
    x1a, y1a, x2a, y2a = _zoom(np.asarray(rois_1))
    x1b, y1b, x2b, y2b = _zoom(np.asarray(rois_2))
    ux1 = np.minimum(x1a, x1b)
    uy1 = np.minimum(y1a, y1b)
    ux2 = np.maximum(x2a, x2b)
    uy2 = np.maximum(y2a, y2b)
    geoms = []
    for b in range(len(ux1)):
        lo_y, hi_y = int(uy1[b]), int(uy2[b])
        lo_x, hi_x = int(ux1[b]), int(ux2[b])
        h = hi_y - lo_y + 1
        w = hi_x - lo_x + 1
        mask = np.zeros((h, w), bool)
        mask[y1a[b] - lo_y:y2a[b] - lo_y + 1, x1a[b] - lo_x:x2a[b] - lo_x + 1] = True
        mask[y1b[b] - lo_y:y2b[b] - lo_y + 1, x1b[b] - lo_x:x2b[b] - lo_x + 1] = True
        rs, hgt = _bin_edges(lo_y, h)
        cs, wdt = _bin_edges(lo_x, w)
        iruns = _runs(rs, hgt)
        jruns = _runs(cs, wdt)
        # cost (DVE cycles): one-stage = grid cells + overhead per run pair;
        # two-stage = x-pass cells + y-pass cells + overhead per run.
        OVH = 75
        one = sum(ni * hgt_ for (_, ni, _, hgt_) in iruns) \
            * sum(nj * wdt_ for (_, nj, _, wdt_) in jruns) \
            + OVH * len(iruns) * len(jruns)
        xcells = h * sum(nj * wdt_ for (_, nj, _, wdt_) in jruns)
        ycells = PW * sum(ni * hgt_ for (_, ni, _, hgt_) in iruns)
        two = xcells + ycells + OVH * (len(iruns) + len(jruns))
        geoms.append(dict(
            uy1=lo_y, uy2=hi_y, ux1=lo_x, ux2=hi_x, h=h, w=w,
            rects=_complement_rects(mask),
            iruns=iruns, jruns=jruns, rs=rs, cs=cs,
            cost=min(one, two) + 250, two_stage=two < one,
        ))
    return geoms


# ------------------------------------------------------------ program build

def _build_core_program(geoms, ylo, nrows):
    import concourse.bacc as bacc
    import concourse.bass as bass
    import concourse.tile as tile
    from concourse import mybir

    f32 = mybir.dt.float32
    nroi = len(geoms)
    nc = bacc.Bacc("TRN2", target_bir_lowering=False, debug=False)
    feat_d = nc.dram_tensor("feat", [C, nrows * W], f32, kind="ExternalInput").ap()
    out_d = nc.dram_tensor("out", [C, nroi * PH * PW], f32, kind="ExternalOutput").ap()

    maxhw = max((g["h"] * g["w"] for g in geoms if g["rects"]), default=64)
    maxth = max((g["h"] for g in geoms if g["two_stage"]), default=1)

    def sub_ap(base, off, dims):
        p0 = list(list(base.ap)[0])
        return bass.AP(base.tensor, base.offset + off, [p0] + [list(d) for d in dims])

    with tile.TileContext(nc) as tc:
        with tc.tile_pool(name="main", bufs=1) as pool, \
             tc.tile_pool(name="vals", bufs=4) as vpool:
            feat_t = pool.tile([C, nrows * W], f32)
            o_t = pool.tile([C, nroi * PH * PW], f32)
            # chunked feature-map DMA so early ROIs can start sooner
            bounds = sorted({0, nrows} | {
                min(nrows, max(0, (nrows * t) // DMA_CHUNKS))
                for t in range(1, DMA_CHUNKS)})
            for r0, r1 in zip(bounds[:-1], bounds[1:]):
                if r1 > r0:
                    nc.sync.dma_start(feat_t[:, r0 * W:r1 * W],
                                      feat_d[:, r0 * W:r1 * W])
            for k, g in enumerate(geoms):
                h, w = g["h"], g["w"]
                rs, cs = g["rs"], g["cs"]
                if g["rects"]:
                    vt = vpool.tile([C, maxhw], f32, tag="v")
                    win = sub_ap(feat_t[:], (g["uy1"] - ylo) * W + g["ux1"],
                                 [[W, h], [1, w]])
                    nc.scalar.copy(
                        vt[:, 0:h * w].rearrange("p (a b) -> p a b", a=h), win)
                    for (r0, r1, c0, c1) in g["rects"]:
                        nc.gpsimd.memset(
                            sub_ap(vt[:], r0 * w + c0, [[w, r1 - r0], [1, c1 - c0]]),
                            0.0)
                    src, pitch, oy, ox = vt[:], w, g["uy1"], g["ux1"]
                else:
                    src, pitch, oy, ox = feat_t[:], W, ylo, 0
                if g["two_stage"]:
                    tt = vpool.tile([C, maxth * PW], f32, tag="t")
                    for (j0, nj, gj, wdt) in g["jruns"]:
                        in_ap = sub_ap(
                            src, (g["uy1"] - oy) * pitch + (int(cs[j0]) - ox),
                            [[pitch, h], [gj, nj], [1, wdt]])
                        out_ap = sub_ap(tt[:], j0, [[PW, h], [1, nj]])
                        nc.vector.tensor_reduce(
                            out_ap, in_ap,
                            axis=mybir.AxisListType.X, op=mybir.AluOpType.max)
                    for (i0, ni, gi, hgt) in g["iruns"]:
                        in_ap = sub_ap(
                            tt[:], (int(rs[i0]) - g["uy1"]) * PW,
                            [[gi * PW, ni], [1, PW], [PW, hgt]])
                        out_ap = sub_ap(o_t[:], k * PH * PW + i0 * PW,
                                        [[PW, ni], [1, PW]])
                        nc.vector.tensor_reduce(
                            out_ap, in_ap,
                            axis=mybir.AxisListType.X, op=mybir.AluOpType.max)
                else:
                    for (i0, ni, gi, hgt) in g["iruns"]:
                        for (j0, nj, gj, wdt) in g["jruns"]:
                            in_ap = sub_ap(
                                src,
                                (int(rs[i0]) - oy) * pitch + (int(cs[j0]) - ox),
                                [[gi * pitch, ni], [gj, nj], [pitch, hgt], [1, wdt]])
                            out_ap = sub_ap(
                                o_t[:], k * PH * PW + i0 * PW + j0,
                                [[PW, ni], [1, nj]])
                            nc.vector.tensor_reduce(
                                out_ap, in_ap,
                                axis=mybir.AxisListType.XY, op=mybir.AluOpType.max)
            half = (nroi // 2) * PH * PW
            nc.sync.dma_start(out_d[:, 0:half], o_t[:, 0:half])
            nc.sync.dma_start(out_d[:, half:], o_t[:, half:])
    nc.compile()
    return nc


# ---------------------------------------------------------------- top level

def _partition_balanced(geoms):
    """Split y-sorted ROIs into 8 contiguous groups minimizing max cost."""
    order = sorted(range(NROIS), key=lambda b: geoms[b]["uy1"] + geoms[b]["uy2"])
    costs = [geoms[b]["cost"] for b in order]
    pre = np.concatenate([[0], np.cumsum(costs)])

    def group_cost(i, j):
        return pre[j] - pre[i]

    n = NROIS
    INF = float("inf")
    dp = np.full((NCORES + 1, n + 1), INF)
    cut = np.zeros((NCORES + 1, n + 1), np.int64)
    dp[0, 0] = 0.0
    for gidx in range(1, NCORES + 1):
        for j in range(gidx, n + 1):
            best, barg = INF, gidx - 1
            for i in range(gidx - 1, j):
                v = max(dp[gidx - 1, i], group_cost(i, j))
                if v < best:
                    best, barg = v, i
            dp[gidx, j] = best
            cut[gidx, j] = barg
    cuts = [n]
    j = n
    for gidx in range(NCORES, 0, -1):
        j = int(cut[gidx, j])
        cuts.append(j)
    cuts = cuts[::-1]
    return [order[cuts[c]:cuts[c + 1]] for c in range(NCORES)]


def _prepare(feature_map, rois_1, rois_2):
    geoms = _geometry(rois_1, rois_2)
    groups = _partition_balanced(geoms)
    fm = np.ascontiguousarray(np.asarray(feature_map), np.float32)[0]  # [C,H,W]
    programs, in_maps, core_ids = [], [], []
    for c in range(NCORES):
        ids = sorted(groups[c], key=lambda b: geoms[b]["uy2"])
        core_geoms = [geoms[b] for b in ids]
        ylo = min(g["uy1"] for g in core_geoms)
        yhi = max(g["uy2"] for g in core_geoms) + 1
        nrows = yhi - ylo
        programs.append(_build_core_program(core_geoms, ylo, nrows))
        in_maps.append({"feat": np.ascontiguousarray(
            fm[:, ylo:yhi, :]).reshape(C, nrows * W)})
        core_ids.append(ids)
    return programs, in_maps, core_ids


def _assemble(outs, core_ids):
    full = np.empty((NROIS, C, PH, PW), np.float32)
    for c in range(NCORES):
        nroi = len(core_ids[c])
        r = outs[c]["out"].reshape(C, nroi, PH, PW).transpose(1, 0, 2, 3)
        for k, b in enumerate(core_ids[c]):
            full[b] = r[k]
    return full


def kernel(feature_map, rois_1, rois_2):
    import jax
    from concourse import bass2jax
    from concurrent.futures import ThreadPoolExecutor

    programs, in_maps, core_ids = _prepare(feature_map, rois_1, rois_2)
    bass2jax.install_neuronx_cc_hook()
    devices = jax.devices()

    def run_one(c):
        with jax.default_device(devices[c]):
            return bass2jax.run_bass_via_pjrt(programs[c], [in_maps[c]], n_cores=1)[0]

    with ThreadPoolExecutor(NCORES) as ex:
        outs = list(ex.map(run_one, range(NCORES)))
    return _assemble(outs, core_ids)
